# revision 17
# baseline (speedup 1.0000x reference)
"""Trainium2 Bass kernel for nn_Decoder_22196390985918 (SPADE-style decoder).

Sharding: 8 cores = (batch b in 0..3) x (H-half in 0..1). Each core computes
out[b, :, h0:h0+64, :] for h0 = 64*(core%2).

Key algorithmic transform: the [B, 512, H, W] "middle" tensor (masked scatter
of per-region style vectors mu[b,j,:]) is never materialized. Since
middle[b,:,h,w] = mu[b, j*(h,w), :] with j* the last active region,
conv(middle) collapses to a conv over the 5 one-hot region masks sel_j with
per-batch tap tables G[j, cc, tap] = sum_k Wconv[cc, k, tap] * mu[b, j, k].

Bulk tensors in bf16 (masks are 0/1 so exact; tolerance 2e-2), weight-table
transposes done host-side (layout only), mu via PE matmuls, one-hot region
selection via sel = relu(seg - cnt) in a single matmul+relu, sigmoid blending
folded into muT (avg branch) and spT (spade branch), small constants packed
into two DMAs, x normalized once up front, bf16 output.
"""
import numpy as np
import ml_dtypes

import concourse.bacc as bacc
import concourse.bass as bass
import concourse.mybir as mybir
import concourse.tile as tile
from concourse.bass_utils import run_bass_kernel_spmd

dt = mybir.dt
F32 = dt.float32
BF16 = dt.bfloat16
AF = mybir.ActivationFunctionType
ALU = mybir.AluOpType
BF = ml_dtypes.bfloat16
F8 = ml_dtypes.float8_e4m3
FP8 = dt.float8e4

B, C, H, W, F, L, NH = 4, 64, 128, 128, 5, 512, 128
GW = 130                    # padded grid width  (image col = grid col - 1)
SR = 66                     # seg/sel/actv grid rows (image row = h0 - 1 + r)
MR = 68                     # mask grid rows (image row = h0 - 2 + r)
SEG_N = SR * GW             # 8580
MASK_N = MR * GW            # 8840
SEG_PAD = SEG_N + 264       # im2col window slack
MASK_PAD = MASK_N + 264
ROWS = 64                   # output rows per core
NCH = 16                    # main conv chunks (4 rows x 128 cols, N=512)
NCORES = 8

# constb (bf16) column offsets
O_U5 = 0            # [45, 45]
O_SSW = 45          # [27, 128]
O_ID5 = 173         # [5, 5]
O_ONE = 178         # [1, 128]
O_HP = 306          # [2, 128]
O_BGBB = 434        # [2, 1]
O_SHUF = 435        # [5, 9*45]
O_EF = 840          # [1, 25]
O_FCB = 865         # [1, 2560]
O_CODE = 3425       # [128, 100]
O_ZERO = 3525       # [128, 132]
O_ID64 = 3657       # [128, 64]: row 64+j has 1 at col j
NB_COLS = 3721
# constf (f32) columns: 0 graw 1 braw 2 convb 3 spadeb 4 ssb 5-6 hal 7 ones
# 8 eps 9 half1
NF_COLS = 10


def _build_nc():
    nc = bacc.Bacc()

    xrd = nc.dram_tensor("xr", [C, H * W], BF16, kind="ExternalInput")
    segg = nc.dram_tensor("segg", [F, SEG_PAD], BF16, kind="ExternalInput")
    maskg = nc.dram_tensor("maskg", [3, MASK_PAD], BF16, kind="ExternalInput")
    codes8 = nc.dram_tensor("codes8", [128, 4 * F * F], BF16,
                            kind="ExternalInput")
    bigd = nc.dram_tensor("bigw", [128, 16000 + NB_COLS - O_SHUF], BF16,
                          kind="ExternalInput")
    cbd = nc.dram_tensor("constb", [128, NB_COLS], BF16, kind="ExternalInput")
    cfd = nc.dram_tensor("constf", [128, NF_COLS], F32, kind="ExternalInput")
    out_d = nc.dram_tensor("out", [C, NCH, 512], BF16, kind="ExternalOutput")

    with tile.TileContext(nc) as tc:
        with (
            tc.tile_pool(name="const", bufs=1) as cst,
            tc.tile_pool(name="gb", bufs=4) as gbp,
            tc.tile_pool(name="pbp", bufs=4) as pbp,
            tc.tile_pool(name="pbs", bufs=3) as pbsp,
            tc.tile_pool(name="ot", bufs=4) as otp,
            tc.tile_pool(name="sq", bufs=2) as sqp,
            tc.tile_pool(name="pmain", bufs=3, space="PSUM") as pmain,
            tc.tile_pool(name="paux", bufs=2, space="PSUM") as paux,
            tc.tile_pool(name="pgp", bufs=3, space="PSUM") as pgp,
        ):
            # ---- sync queue: early consts, fcwT, late consts, wct -------
            cba = cst.tile([128, O_SHUF], BF16)
            nc.sync.dma_start(out=cba[:], in_=cbd[:, 0:O_SHUF])
            cf = cst.tile([128, NF_COLS], F32)
            nc.sync.dma_start(out=cf[:], in_=cfd[:])
            codes_sb = cst.tile([128, 4, F, F], BF16)
            nc.sync.dma_start(out=codes_sb[:], in_=codes8[:].rearrange(
                "p (l f m) -> p l f m", l=4, f=F))
            # three tiles, loaded smallest/earliest-needed first, so
            # taps (spT), mu (fcw) and G (wct) unblock independently
            t_tail = cst.tile([128, 1152 + NB_COLS - O_SHUF], BF16)
            nc.sync.dma_start(out=t_tail[:], in_=bigd[:, 14848:])
            fcw_h = []
            for h in range(2):
                t = cst.tile([128, 2, F, L], BF16, name=f"fcwh{h}")
                nc.sync.dma_start(out=t[:], in_=bigd[:, h * 5120:(h + 1) * 5120]
                                  .rearrange("p (a f l) -> p a f l", a=2, f=F))
                fcw_h.append(t)
            t_wct = cst.tile([128, 4608], BF16)
            nc.sync.dma_start(out=t_wct[:], in_=bigd[:, 10240:14848])

            # ---- scalar queue: grids, spT, wct --------------------------
            sel45 = cst.tile([45, SEG_N], BF16)
            segp = segg[:].ap[0][0]
            for ty in range(3):
                src = bass.AP(tensor=segg[:].tensor, offset=ty * GW,
                              ap=[[1, 3], [segp, F], [1, SEG_N]])
                nc.scalar.dma_start(out=sel45[15 * ty:15 * ty + 15, :],
                                    in_=src)
            mask27 = cst.tile([27, MASK_N], BF16)
            maskp_ = maskg[:].ap[0][0]
            for ty in range(3):
                src = bass.AP(tensor=maskg[:].tensor, offset=ty * GW,
                              ap=[[1, 3], [maskp_, 3], [1, MASK_N]])
                nc.scalar.dma_start(out=mask27[9 * ty:9 * ty + 9, :], in_=src)
            wct_all = t_wct[:].rearrange(
                "p (a t c) -> p a t c", a=4, t=9)
            spTv = t_tail[:, 0:1152].rearrange("p (t c) -> p t c", t=9)

            # ---- gpsimd queue: x quarters -------------------------------
            xqs = []
            for q in range(4):
                xq = cst.tile([C, 4096], BF16, name=f"xq{q}")
                nc.gpsimd.dma_start(out=xq[:],
                                    in_=xrd[:, q * 4096:(q + 1) * 4096])
                xqs.append(xq)

            # ---- const views --------------------------------------------
            u5v = cba[0:45, O_U5:O_U5 + 45]
            sswv = cba[0:27, O_SSW:O_SSW + 128]
            id5v = cba[0:5, O_ID5:O_ID5 + 5]
            one128v = cba[0:1, O_ONE:O_ONE + 128]
            hpv = cba[0:2, O_HP:O_HP + 128]
            bgbbv = cba[0:2, O_BGBB:O_BGBB + 1]
            shufv = t_tail[0:5, 1152:1557].rearrange("p (t m) -> p t m", t=9)
            efv = t_tail[0:1, 1152 + O_EF - O_SHUF:1152 + O_EF - O_SHUF + 25].rearrange(
                "p (f m) -> p f m", f=F)
            fcbv = t_tail[0:1, 1152 + O_FCB - O_SHUF:1152 + O_FCB - O_SHUF + 2560].rearrange(
                "p (f l) -> p f l", f=F)
            id64v = t_tail[:, 1152 + O_ID64 - O_SHUF:1152 + O_ID64 - O_SHUF + 64]
            zerov = cst.tile([128, 132], BF16)
            nc.gpsimd.memset(zerov[:], 0.0)
            grawv = cf[:, 0:1]
            brawv = cf[:, 1:2]
            convbv = cf[:, 2:3]
            spadebv = cf[:, 3:4]
            ssbv = cf[:, 4:5]
            halv = cf[:, 5:7]
            onesv = cf[:, 7:8]
            epsv = cf[0:64, 8:9]
            half1v = cf[:, 9:10]

            # ---- blending factors ---------------------------------------
            gsig = cst.tile([128, 1], F32)
            nc.scalar.activation(gsig[:], grawv, AF.Sigmoid)
            bsig = cst.tile([128, 1], F32)
            nc.scalar.activation(bsig[:], brawv, AF.Sigmoid)
            omg_t = cst.tile([128, 1], F32)
            nc.scalar.activation(omg_t[:], gsig[:], AF.Identity,
                                 bias=onesv, scale=-1.0)
            omb_t = cst.tile([128, 1], F32)
            nc.scalar.activation(omb_t[:], bsig[:], AF.Identity,
                                 bias=onesv, scale=-1.0)
            # spade branch tables scaled by (1-sigmoid): gamma cols 0-63 by
            # 1-ga, beta cols 64-127 by 1-ba (constant per-partition scale)
            # 1-sigmoid broadcast tile [128, 128] (per out-channel cc)
            praw = pgp.tile([1, 128], F32, tag="g")
            nc.tensor.matmul(praw[:], bgbbv, hpv, start=True, stop=True)
            savg_row = cst.tile([1, 128], BF16)
            nc.scalar.activation(savg_row[:], praw[:], AF.Sigmoid)
            ssp_row = cst.tile([1, 128], BF16)
            nc.scalar.activation(ssp_row[:], savg_row[:], AF.Identity,
                                 bias=onesv[0:1, :], scale=-1.0)
            psp = pgp.tile([128, 128], F32, tag="g")
            nc.tensor.matmul(psp[:], one128v, ssp_row[:], start=True,
                             stop=True)
            ssp_tile = cst.tile([128, 128], BF16)
            nc.scalar.activation(ssp_tile[:], psp[:], AF.Copy)
            spT = cst.tile([128, 9, 128], BF16)
            for t in range(9):
                nc.vector.tensor_mul(spT[:, t, :], spTv[:, t, :], ssp_tile[:])

            # ---- instance-norm stats (DVE) ------------------------------
            stats_t = cst.tile([C, 32, 6], F32)
            for q in range(4):
                xv = xqs[q][:].rearrange("c (k n) -> c k n", n=512)
                for k in range(8):
                    nc.vector.bn_stats(out=stats_t[:, 8 * q + k, :],
                                       in_=xv[:, k, :])
            mv = cst.tile([C, 2], F32)
            nc.vector.bn_aggr(out=mv[:], in_=stats_t[:])
            sd = cst.tile([C, 1], F32)
            nc.scalar.activation(sd[:], mv[:, 1:2], AF.Sqrt,
                                 bias=epsv, scale=1.0)
            rstd = cst.tile([C, 1], F32)
            nc.vector.reciprocal(rstd[:], sd[:])
            nbias = cst.tile([C, 1], F32)
            nc.vector.tensor_mul(nbias[:], mv[:, 0:1], rstd[:])
            nc.vector.tensor_scalar_mul(nbias[:], nbias[:], -1.0)
            # normalize own half once: xn = x*rstd + nbias (bf16)
            xnq = []
            for q in range(2):
                xn = cst.tile([C, 4096], BF16, name=f"xn{q}")
                nc.vector.tensor_scalar(xn[:], xqs[q][:], rstd[:], nbias[:],
                                        op0=ALU.mult, op1=ALU.add)
                xnq.append(xn)

            # actv tile + border zeroing (DVE, early)
            actv = cst.tile([NH, SR, GW], BF16)
            bord = actv[:, :, 0:1]
            nc.vector.tensor_copy(
                bass.AP(tensor=bord.tensor, offset=bord.offset,
                        ap=[bord.ap[0], [GW, SR], [GW - 1, 2]]),
                zerov[:].rearrange("p (a b) -> p a b", a=SR))

            # ---- region masks: sel = relu(seg - cnt) --------------------
            sel2 = cst.tile([45, SEG_N], BF16)
            segchunks = []
            off = 0
            while off < SEG_N:
                n = min(512, SEG_N - off)
                segchunks.append((off, n))
                off += n
            for ci, (off, n) in enumerate(segchunks):
                pc = paux.tile([45, 512], F32, tag="aux")
                nc.tensor.matmul(pc[:, 0:n], u5v, sel45[:, off:off + n],
                                 start=True, stop=True)
                nc.scalar.activation(sel2[:, off:off + n], pc[:, 0:n],
                                     AF.Relu)

            # ---- shared conv (mask 3 -> NH): 17 chunks of <=4 rows ------
            m3 = mask27[:].rearrange("p (r c) -> p r c", c=GW)
            achunks = [(4 * a, 4) for a in range(16)] + [(64, 2)]
            for ai, (r, nr) in enumerate(achunks):
                psh = paux.tile([NH, 4, 128], F32, tag="aux")
                nc.tensor.matmul(psh[:, 0:nr, :], sswv,
                                 m3[:, r:r + nr, 0:128], start=True, stop=True)
                nc.scalar.activation(actv[:, r:r + nr, 1:129],
                                     psh[:, 0:nr, :], AF.Relu,
                                     bias=ssbv, scale=1.0)
            # zero/keep halo rows (outside image for edge cores)
            nc.gpsimd.tensor_scalar_mul(actv[:, 0, :], actv[:, 0, :],
                                        halv[:, 0:1])
            nc.gpsimd.tensor_scalar_mul(actv[:, SR - 1, :], actv[:, SR - 1, :],
                                        halv[:, 1:2])

            # ---- epilogue blending biases (DVE, cheap) ------------------
            om_gba = cst.tile([128, 1], F32)
            gba = cst.tile([128, 1], F32)
            nc.vector.tensor_copy(gba[0:64, :], gsig[0:64, :])
            nc.vector.tensor_copy(gba[64:128, :], bsig[64:128, :])
            nc.scalar.activation(om_gba[:], gba[:], AF.Identity,
                                 bias=onesv, scale=-1.0)
            tb1 = cst.tile([128, 1], F32)
            nc.vector.tensor_mul(tb1[:], convbv, gba[:])
            tb2 = cst.tile([128, 1], F32)
            nc.vector.tensor_mul(tb2[:], spadebv, om_gba[:])
            bias_t = cst.tile([128, 1], F32)
            nc.vector.tensor_add(bias_t[:], tb1[:], tb2[:])
            bias1_t = cst.tile([128, 1], F32)
            nc.vector.tensor_add(bias1_t[:], bias_t[:], half1v)

            # ---- main conv chunks + epilogue ----------------------------
            s3 = sel2[:].rearrange("p (r c) -> p r c", c=GW)
            pms = {}

            def conv_chunk(i):
                pm = pmain.tile([128, 4, 128], F32, tag="pm", name=f"pm_{i}")
                pms[i] = pm
                for t in range(9):
                    ty, tx = divmod(t, 3)
                    nc.tensor.matmul(pm[:], spT[:, t, :],
                                     actv[:, 4 * i + ty:4 * i + ty + 4,
                                          tx:tx + 128],
                                     start=(t == 0), stop=False)

            def close_chunk(i):
                pm = pms[i]
                nc.tensor.matmul(pm[:], selG[:], s3[:, 4 * i:4 * i + 4, 0:128],
                                 start=False, stop=True)

            def epi_chunk2(i):
                pm = pms.pop(i)
                gb = gbp.tile([128, 4, 128], BF16, tag="gb", name=f"gb_{i}")
                nc.scalar.activation(gb[:], pm[:], AF.Identity,
                                     bias=bias1_t[:], scale=1.0)
                # beta rows -> partitions 0-63 via PE shift matmul
                pool = pgp if i % 2 == 0 else paux
                pb = pool.tile([64, 4, 128], F32,
                               tag="g" if i % 2 == 0 else "aux",
                               name=f"pb_{i}")
                nc.tensor.matmul(pb[:].rearrange("p t c -> p (t c)"), id64v,
                                 gb[:].rearrange("p t c -> p (t c)"),
                                 start=True, stop=True)
                pbs = pbsp.tile([64, 4, 128], F32, tag="pbs",
                                name=f"pbs_{i}")
                nc.scalar.activation(pbs[:], pb[:], AF.Copy)
                xn = xnq[i // 8][:, (i % 8) * 512:(i % 8 + 1) * 512].rearrange(
                    "c (r w) -> c r w", r=4)
                xnt = otp.tile([C, 4, 128], F32, tag="ot", name=f"xnt_{i}")
                nc.gpsimd.tensor_mul(xnt[:], xn, gb[0:64, :, :])
                obf = pbp.tile([64, 4, 128], BF16, tag="pb", name=f"ob_{i}")
                nc.vector.tensor_add(obf[:].rearrange("p t c -> p (t c)"),
                                     xnt[:].rearrange("p t c -> p (t c)"),
                                     pbs[:].rearrange("p t c -> p (t c)"))
                nc.sync.dma_start(out=out_d[:, i, :],
                                  in_=obf[:].rearrange("c r w -> c (r w)"))

            conv_chunk(0)
            conv_chunk(1)
            conv_chunk(2)
            # ---- mu = relu(codes @ fcw^T + fc_b) on PE ------------------
            # codev[:, lb, f, :] is one-hot in column f, so each matmul
            # contributes only row f of the [F, L] z accumulator.
            pz = pgp.tile([F, L], F32, tag="g", name="pz")
            for lb in range(4):
                for f in range(F):
                    nc.tensor.matmul(pz[:], codes_sb[:, lb, f, :],
                                     fcw_h[lb // 2][:, lb % 2, f, :],
                                     start=(lb == 0 and f == 0), stop=False)
            for f in range(F):
                nc.tensor.matmul(pz[:], efv[:, f, :], fcbv[:, f, :],
                                 start=False, stop=(f == F - 1))
            mur = cst.tile([F, L], BF16)
            nc.scalar.activation(mur[:], pz[:], AF.Relu)
            muT = cst.tile([128, 4, F], BF16)
            for kb in range(4):
                ptm = paux.tile([128, F], BF16, tag="aux")
                nc.tensor.transpose(ptm[:], mur[:, kb * 128:(kb + 1) * 128],
                                    id5v)
                nc.scalar.activation(muT[:, kb, :], ptm[:], AF.Copy)
            # ---- G tables -> selG ---------------------------------------
            gps = [pgp.tile([F, 3, 128], F32, tag="g", name=f"gps{g}")
                   for g in range(3)]
            for kb in range(4):
                for g in range(3):
                    nc.tensor.matmul(gps[g][:], muT[:, kb, :],
                                     wct_all[:, kb, 3 * g:3 * g + 3, :],
                                     start=(kb == 0), stop=(kb == 3))
            gstage = cst.tile([F, 9, 128], BF16)
            for g in range(3):
                nc.scalar.activation(gstage[:, 3 * g:3 * g + 3, :], gps[g][:],
                                     AF.Copy)
            # partition shuffle [f, t] -> row 5t+f via 9 tiny matmuls
            selG_ps = paux.tile([45, 128], F32, tag="aux")
            for t in range(9):
                nc.tensor.matmul(selG_ps[:], shufv[:, t, :], gstage[:, t, :],
                                 start=(t == 0), stop=(t == 8))
            # avg branch scaled by sigmoid: gamma cols by ga, beta by ba
            selG = cst.tile([45, 128], BF16)
            nc.scalar.activation(selG[:, 0:64], selG_ps[:, 0:64], AF.Identity,
                                 scale=gsig[0:45, :])
            nc.scalar.activation(selG[:, 64:128], selG_ps[:, 64:128],
                                 AF.Identity, scale=bsig[0:45, :])

            for i in range(3, NCH):
                close_chunk(i - 3)
                epi_chunk2(i - 3)
                conv_chunk(i)
            for i in range(NCH - 3, NCH):
                close_chunk(i)
                epi_chunk2(i)

    nc.finalize()
    return nc


_NC = None


def kernel(**inputs):
    global _NC
    x = np.asarray(inputs["x"], dtype=np.float32)
    segmap = np.asarray(inputs["segmap"], dtype=np.float32)
    codes_vector = np.asarray(inputs["codes_vector"], dtype=np.float32)
    mask = np.asarray(inputs["mask"], dtype=np.float32)
    fc_w = np.asarray(inputs["fc_w"], dtype=np.float32)
    fc_b = np.asarray(inputs["fc_b"], dtype=np.float32)
    conv_gamma_w = np.asarray(inputs["conv_gamma_w"], dtype=np.float32)
    conv_gamma_b = np.asarray(inputs["conv_gamma_b"], dtype=np.float32)
    conv_beta_w = np.asarray(inputs["conv_beta_w"], dtype=np.float32)
    conv_beta_b = np.asarray(inputs["conv_beta_b"], dtype=np.float32)
    spade_shared_w = np.asarray(inputs["spade_shared_w"], dtype=np.float32)
    spade_shared_b = np.asarray(inputs["spade_shared_b"], dtype=np.float32)
    spade_gamma_w = np.asarray(inputs["spade_gamma_w"], dtype=np.float32)
    spade_gamma_b = np.asarray(inputs["spade_gamma_b"], dtype=np.float32)
    spade_beta_w = np.asarray(inputs["spade_beta_w"], dtype=np.float32)
    spade_beta_b = np.asarray(inputs["spade_beta_b"], dtype=np.float32)
    blending_gamma = np.asarray(inputs["blending_gamma"], dtype=np.float32)
    blending_beta = np.asarray(inputs["blending_beta"], dtype=np.float32)

    if _NC is None:
        _NC = _build_nc()

    # host-side layout prep (transposes / packing / dtype cast only)
    # fcwT[p, lb, f, kout] = fc_w[f, kout, lb*128+p]
    fcwT_h = np.ascontiguousarray(
        fc_w.transpose(2, 0, 1).reshape(4, 128, F, L).transpose(1, 0, 2, 3)
        .reshape(128, 4 * F * L)).astype(BF)
    # wct[kb*128+p, t, cc] = w_cat[cc, kb*128+p, t], cc = gamma||beta
    wcat = np.concatenate([conv_gamma_w.reshape(C, L, 9),
                           conv_beta_w.reshape(C, L, 9)], axis=0)
    wct_h = np.ascontiguousarray(
        wcat.transpose(1, 2, 0).reshape(4 * 128, 9 * 128)).astype(BF)
    scat = np.concatenate([spade_gamma_w.reshape(C, NH, 9),
                           spade_beta_w.reshape(C, NH, 9)], axis=0)
    spt_h = np.ascontiguousarray(
        scat.transpose(1, 2, 0).reshape(128, 9 * 128)).astype(BF)

    # packed bf16 constants (shared part)
    cbase = np.zeros((128, NB_COLS), np.float32)
    # u5' such that u5'.T @ seg45 = seg - (count of later active regions)
    u5p = np.kron(np.eye(9, dtype=np.float32),
                  np.eye(F, dtype=np.float32)
                  - np.tril(np.ones((F, F), np.float32), -1))
    cbase[0:45, O_U5:O_U5 + 45] = u5p
    cbase[0:27, O_SSW:O_SSW + 128] = \
        spade_shared_w.transpose(2, 3, 1, 0).reshape(27, NH)
    cbase[0:5, O_ID5:O_ID5 + 5] = np.eye(5)
    cbase[0:1, O_ONE:O_ONE + 128] = 1.0
    cbase[0, O_HP:O_HP + 64] = 1.0
    cbase[1, O_HP + 64:O_HP + 128] = 1.0
    cbase[0, O_BGBB] = blending_gamma[0]
    cbase[1, O_BGBB] = blending_beta[0]
    shuf = np.zeros((F, 9, 45), np.float32)
    for t in range(9):
        for f in range(F):
            shuf[f, t, 5 * t + f] = 1.0
    cbase[0:5, O_SHUF:O_SHUF + 405] = shuf.reshape(F, 405)
    cbase[0:1, O_EF:O_EF + 25] = np.eye(F).reshape(1, 25)
    cbase[0:1, O_FCB:O_FCB + 2560] = fc_b.reshape(1, F * L)
    cbase[64:128, O_ID64:O_ID64 + 64] = np.eye(64)

    # packed f32 constants (shared part)
    cfbase = np.zeros((128, NF_COLS), np.float32)
    cfbase[:, 0] = blending_gamma[0]
    cfbase[:, 1] = blending_beta[0]
    cfbase[0:64, 2] = conv_gamma_b
    cfbase[64:128, 2] = conv_beta_b
    cfbase[0:64, 3] = spade_gamma_b
    cfbase[64:128, 3] = spade_beta_b
    cfbase[:, 4] = spade_shared_b
    cfbase[:, 7] = 1.0
    cfbase[:, 8] = 1e-5
    cfbase[0:64, 9] = 1.0

    wct_r = np.ascontiguousarray(
        wct_h.reshape(4, 128, 1152).transpose(1, 0, 2).reshape(128, 4608))
    big_shared = np.concatenate([fcwT_h, wct_r, spt_h], axis=1)
    shared = {}

    in_maps = []
    for c in range(NCORES):
        b, half = divmod(c, 2)
        h0 = half * ROWS
        segp = np.zeros((F, SEG_PAD), np.float32)
        segp2 = np.zeros((F, SR, GW), np.float32)
        r_lo, r_hi = h0 - 1, h0 + ROWS + 1  # exclusive
        s_lo, s_hi = max(r_lo, 0), min(r_hi, H)
        segp2[:, s_lo - r_lo:s_hi - r_lo, 1:129] = segmap[b, :, s_lo:s_hi, :]
        segp[:, 0:SEG_N] = segp2.reshape(F, -1)
        maskp = np.zeros((3, MASK_PAD), np.float32)
        maskp2 = np.zeros((3, MR, GW), np.float32)
        m_lo, m_hi = h0 - 2, h0 + ROWS + 2
        ms_lo, ms_hi = max(m_lo, 0), min(m_hi, H)
        maskp2[:, ms_lo - m_lo:ms_hi - m_lo, 1:129] = mask[b, :, ms_lo:ms_hi, :]
        maskp[:, 0:MASK_N] = maskp2.reshape(3, -1)
        # x, own half first (epilogue reads quarters 0-1; stats read all)
        xr = np.concatenate([x[b, :, h0:h0 + ROWS, :],
                             x[b, :, ROWS - h0:H - h0, :]], axis=1)
        # codesT5[p, lb, f, m] = codes[f, lb*128+p] * (m == f)
        cT = codes_vector[b].T.reshape(4, 128, F).transpose(1, 0, 2)
        c5 = cT[:, :, :, None] * np.eye(F, dtype=np.float32)[None, None]
        cbc = cbase
        cfc = cfbase.copy()
        cfc[:, 5] = 0.0 if h0 == 0 else 1.0
        cfc[:, 6] = 0.0 if h0 + ROWS == H else 1.0
        bigw = np.concatenate(
            [big_shared, cbc.astype(BF)[:, O_SHUF:NB_COLS]], axis=1)
        in_maps.append(dict(
            shared,
            bigw=np.ascontiguousarray(bigw),
            xr=np.ascontiguousarray(xr.reshape(C, H * W)).astype(BF),
            codes8=c5.reshape(128, 100).astype(BF),
            segg=np.ascontiguousarray(segp).astype(BF),
            maskg=np.ascontiguousarray(maskp).astype(BF),
            constb=cbc.astype(BF),
            constf=cfc,
        ))

    res = run_bass_kernel_spmd(_NC, in_maps, list(range(NCORES)))

    out = np.empty((B, C, H, W), np.float32)
    for c in range(NCORES):
        b, half = divmod(c, 2)
        h0 = half * ROWS
        out[b, :, h0:h0 + ROWS, :] = \
            res.results[c]["out"].astype(np.float32).reshape(C, ROWS, W)
    return out
